# revision 26
# baseline (speedup 1.0000x reference)
"""Trainium2 Bass kernel for nn_DilatedConvModel (retrieval_knn).

Model: eeg [B,T,64] -> 1x1 conv (64->8) -> dilated conv stack (8->16->16->16,
dilations 1,3,9, VALID, relu); stimulus [B,S,T,1] -> dilated stack
(1->16->16->16); cosine similarity between all stim/eeg channel pairs over
time; 256->1 linear.  B=64, S=8, T=8192.

Sharding: pure data parallel over B across 8 cores (8 batches per core).

Per-core dataflow: channel-major convs on PE with block-diagonal weights
over the 8 local sequences.  Conv layers run as fp8e4m3 DoubleRow matmuls
(2 k-subtiles/instruction, 0.5 PE-cycles per moving column): the 3 taps are
packed as subtile pairs (tap0,tap2) [stride 2*dil, even as required] and
(tap1, zero-tap).  Activations are written as fp8 directly by the PSUM
evacuation (bias+relu) pass; buffer tails are memset so the zero-weight
junk reads stay finite.  Cosine dot products and norms stay bf16/f32.
"""

from contextlib import ExitStack

import numpy as np
import ml_dtypes

import concourse.bass as bass
import concourse.tile as tile
from concourse import mybir
from concourse.bass_utils import run_bass_kernel_spmd
from concourse.vector_clock import ScopedClock

# ---------------------------------------------------------------------------
# Workaround for walrus in this container rejecting >1 sync wait per
# instruction ("Too many sync wait commands"): (a) distribute the
# TileContext tail drain's sem waits across sync-engine nops, (b) post-pass
# that hoists extra waits of any instruction onto standalone EventSemaphore
# instructions inserted just before it on the same engine.
# ---------------------------------------------------------------------------
_MAX_WAITS = 1


def _patched_drain_and_barrier(self, tick_clock, wait_clock):
    nc = self.nc
    probe = nc.sync.nop()
    wait_clock.add_sem_waits(probe.ins,
                             ScopedClock({None: tick_clock.global_clock}))
    si = probe.ins.sync_info
    waits = list(si.on_wait) if si and si.on_wait else []
    if len(waits) > _MAX_WAITS:
        si.on_wait = waits[:_MAX_WAITS]
        rest = waits[_MAX_WAITS:]
        while rest:
            extra = nc.sync.nop()
            extra.ins.sync_info = mybir.SyncInfo(on_wait=rest[:_MAX_WAITS],
                                                 on_update=[])
            rest = rest[_MAX_WAITS:]
    # probe/extra nops precede the drain in SP program order, so the drain
    # only runs after every lane's final tick — no waits needed on it.
    nc.sync.drain()
    nc.all_engine_barrier()
    assert self.sems is not None
    popped = nc._tile_sem_poison_stack.pop()
    assert popped is self._sem_poison
    nc.clear_and_free_semaphores(list(self.sems.allocated().values()))
    nc.all_engine_barrier()


def _split_multi_waits(nc, max_waits=_MAX_WAITS):
    f = nc.m.functions[0]
    ctr = 0
    for bb in f.blocks:
        new_insts = []
        for inst in bb.instructions:
            si = inst.sync_info
            waits = list(si.on_wait) if si and si.on_wait else []
            if len(waits) > max_waits:
                for w in waits[:-max_waits]:
                    ev = mybir.InstEventSemaphore(
                        name=f"waitsplit_{ctr}", opcode="EventSemaphore",
                        engine=inst.engine, ins=[], outs=[],
                        sync_info=mybir.SyncInfo(on_wait=[w], on_update=[]))
                    ctr += 1
                    new_insts.append(ev)
                si.on_wait = waits[-max_waits:]
            new_insts.append(inst)
        try:
            bb.instructions[:] = new_insts
        except TypeError:
            bb.instructions = new_insts


tile.TileContext._drain_and_barrier = _patched_drain_and_barrier

BF16 = mybir.dt.bfloat16
F32 = mybir.dt.float32
FP8 = mybir.dt.float8e4
AF = mybir.ActivationFunctionType
ALU = mybir.AluOpType
DRM = mybir.MatmulPerfMode.DoubleRow

B, S, T, C_EEG = 64, 8, 8192, 64
N_CORES = 8
BPC = B // N_CORES          # 8 sequences per core
CH = 512                    # fp32 PSUM chunk width
PAD = 64                    # activation tile tail pad (junk reads of P1 sub1)
L_C1, L_E1, L_E2, L_E3 = 8192, 8190, 8184, 8166
EPS = 1e-8

# per-layer precision: "q8" = fp8 DoubleRow (1.0 cyc/col/layer),
# "bf16" = 3-tap bf16 (3.0 cyc/col/layer).  s1: "fp8" (0.5) or "bf16" (1.0).
# The stim path must stay bf16: fp8 perturbs the st3 direction ~3%
# coherently, which multiplies every cosine (measured 2-3e-2 rel err).
MODE = {"e1": "q8", "e23": "q8", "s1": "bf16", "s23": "bf16"}
SQ_TTR = False

_NC_CACHE = {}


def _act_dt(mode):
    return FP8 if mode in ("q8",) else BF16


def _chunks(length):
    out, t0 = [], 0
    while t0 < length:
        w = min(CH, length - t0)
        out.append((t0, w))
        t0 += w
    return out


def _dr_rhs(src_m, t0, stride, width, parts=128):
    """[parts, 2, width] view of flat [parts, T] tile: subtiles at t0 and
    t0+stride (stride must be even for fp8 DoubleRow)."""
    ap = src_m[0:parts, t0:t0 + 1].copy()
    v = ap.ap
    part = tuple(v[0])
    v.clear()
    v.append(part)
    v.append((stride, 2))
    v.append((1, width))
    return ap


def _const_shapes():
    d = {
        "id128": ((128, 128), BF16),
        "id128f": ((128, 128), F32),
        "W2c2": ((128, 16), F32),
        "ones_blk": ((128, 8), F32),
        "blin": ((1, 1), F32),
    }
    # s1 weights
    if MODE["s1"] == "fp8":
        d["Ws1dr"] = ((12, 2 * 128), FP8)
    else:
        d["Ws1"] = ((24, 128), BF16)
    # e1 fused weights: q8 = per-tap DoubleRow over 2 seq-pair k-subtiles
    # (4 seqs, 64 out cols); bf16 = 2-seq packed 32-col strips
    if MODE["e1"] == "q8":
        for k in range(3):
            d[f"Wf1dr{k}"] = ((128, 2 * 64), FP8)
    else:
        for k in range(3):
            d[f"Wf1_{k}"] = ((128, 32), BF16)
    for pfx, mk in (("We", "e23"), ("Ws", "s23")):
        if MODE[mk] == "q8":
            for l in (2, 3):
                for pi in range(2):
                    d[f"{pfx}{l}p{pi}"] = ((128, 2 * 128), FP8)
        else:
            for l in (2, 3):
                for k in range(3):
                    d[f"{pfx}{l}_{k}"] = ((128, 128), BF16)
    for n in ("bias_e1", "bias_e2", "bias_e3", "bias_s1", "bias_s2",
              "bias_s3"):
        d[n] = ((128, 1), F32)
    return d


def _blob_layout():
    """column layout of consts inside the three blobs (rows, cols, dtype)"""
    items = {"bf": [], "f32": [], "f8": []}
    for name, (shape, dt) in _const_shapes().items():
        key = "bf" if dt == BF16 else ("f32" if dt == F32 else "f8")
        items[key].append((name, shape))
    lay, offs = {}, {"bf": 0, "f32": 0, "f8": 0}
    for key in ("bf", "f32", "f8"):
        for name, shape in items[key]:
            lay[name] = (key, offs[key], shape)
            offs[key] += shape[1]
    return lay, offs


def _build_body(nc, tc, dram):
    eeg_in, stim_in, out_dram = dram["eeg_in"], dram["stim_in"], dram["out"]

    with ExitStack() as ctx:
        const_p = ctx.enter_context(tc.tile_pool(name="const", bufs=1))
        persist_p = ctx.enter_context(tc.tile_pool(name="persist", bufs=1))
        early_p = ctx.enter_context(tc.tile_pool(name="early", bufs=1))
        psC_p = ctx.enter_context(tc.tile_pool(name="psC", bufs=3,
                                               space="PSUM"))

        lay, offs = _blob_layout()
        blobs = {}
        for key, dt, dname in (("bf", BF16, "blob_bf"),
                               ("f32", F32, "blob_f32"),
                               ("f8", FP8, "blob_f8")):
            if offs[key]:
                bt = const_p.tile([128, offs[key]], dt, name=dname)
                nc.sync.dma_start(bt[:], dram[dname][:])
                blobs[key] = bt

        def cload(name):
            which, off, shape = lay[name]
            return blobs[which][0:shape[0], off:off + shape[1]]

        def cload_pair(name, m):
            which, off, shape = lay[name]
            return blobs[which][0:shape[0], off:off + shape[1]].rearrange(
                "p (a m) -> p a m", a=2)

        id128 = cload("id128")
        id128f = cload("id128f")
        bias = {n: cload(n) for n in
                ("bias_e1", "bias_e2", "bias_e3",
                 "bias_s1", "bias_s2", "bias_s3")}
        W2c2 = cload("W2c2")
        ones_blk = cload("ones_blk")
        blin = cload("blin")
        nxr = const_p.tile([1, 128], F32, name="nxr")

        out_sb = const_p.tile([1, BPC * S], F32, name="out_sb")
        inv_nx = const_p.tile([128, 1], F32, name="inv_nx")
        sqscr = const_p.tile([128, 2048], BF16, name="sqscr")

        xT = persist_p.tile([128, T // 128, 128], BF16, name="xT")
        xf = persist_p.tile([128, T], BF16, name="xf")

        evac_ctr = [0]
        EVAC_PAT = "AADADAAD"   # ACT:DVE = 5:3 (Pool cannot read PSUM)

        def evac_relu(dst, src, bias_t):
            c = EVAC_PAT[evac_ctr[0] % len(EVAC_PAT)]
            if c == "A":
                nc.scalar.activation(dst, src, AF.Relu, bias=bias_t[:, 0:1])
            elif c == "D":
                nc.vector.tensor_scalar(dst, src, bias_t[:, 0:1], 0.0,
                                        ALU.add, ALU.max)
            else:
                nc.gpsimd.tensor_scalar(dst, src, bias_t[:, 0:1], 0.0,
                                        ALU.add, ALU.max)
            evac_ctr[0] += 1

        def evac_copy(dst, src):
            c = EVAC_PAT[evac_ctr[0] % len(EVAC_PAT)]
            if c == "A":
                nc.scalar.copy(dst, src)
            elif c == "D":
                nc.vector.tensor_copy(dst, src)
            else:
                nc.gpsimd.tensor_copy(dst, src)
            evac_ctr[0] += 1

        def conv_layer_bf16(src_m, rows, dst_m, out_len, dil, Wk, bn):
            chs = _chunks(out_len)
            for i in range(0, len(chs), 2):
                grp = chs[i:i + 2]
                ps = psC_p.tile([128, 2 * CH], F32, name="psconv",
                                tag="psconv")
                for k in range(3):
                    for j, (t0, w) in enumerate(grp):
                        nc.tensor.matmul(
                            ps[:, j * CH:j * CH + w], Wk[k][:],
                            src_m[0:rows, t0 + k * dil:t0 + k * dil + w],
                            start=(k == 0), stop=(k == 2))
                t0 = grp[0][0]
                wtot = CH + grp[1][1] if len(grp) == 2 else grp[0][1]
                evac_relu(dst_m[:, t0:t0 + wtot], ps[:, :wtot], bias[bn])

        def conv_layer_dr(src_m, dst_m, out_len, dil, Wpairs, bn):
            # Wpairs: [(Wpair_ap, tap_offset), ...]; each a DoubleRow over
            # subtile stride 2*dil.
            chs = _chunks(out_len)
            npi = len(Wpairs)
            for i in range(0, len(chs), 2):
                grp = chs[i:i + 2]
                ps = psC_p.tile([128, 2 * CH], F32, name="psconv",
                                tag="psconv")
                for pi, (Wp, off) in enumerate(Wpairs):
                    for j, (t0, w) in enumerate(grp):
                        nc.tensor.matmul(
                            ps[:, j * CH:j * CH + w], Wp,
                            _dr_rhs(src_m, t0 + off, 2 * dil, w),
                            start=(pi == 0), stop=(pi == npi - 1),
                            perf_mode=DRM)
                t0 = grp[0][0]
                wtot = CH + grp[1][1] if len(grp) == 2 else grp[0][1]
                evac_relu(dst_m[:, t0:t0 + wtot], ps[:, :wtot], bias[bn])

        def conv_layer(src_m, dst_m, out_len, dil, mk, wname, bn):
            if MODE[mk] == "q8":
                Wp = [(cload_pair(f"{wname}p0", 128), 0),
                      (cload_pair(f"{wname}p1", 128), dil)]
                conv_layer_dr(src_m, dst_m, out_len, dil, Wp, bn)
            else:
                Wk = [cload(f"{wname}_{k}") for k in range(3)]
                conv_layer_bf16(src_m, 128, dst_m, out_len, dil, Wk, bn)

        s23_dt = _act_dt(MODE["s23"])
        e23_dt = _act_dt(MODE["e23"])
        e1_dt = _act_dt(MODE["e1"])

        # ---- early: stimulus group 0 s1+s2 (fills PE while eeg DMA runs)
        if MODE["s1"] == "fp8":
            s1mov = early_p.tile([12, 2 * T], FP8, name="s1mov")
        else:
            s1mov = early_p.tile([24, T], BF16, name="s1mov")
        s2in = early_p.tile([128, L_E1 + PAD], s23_dt, name="s2in")
        s3in = early_p.tile([128, L_E2 + PAD], s23_dt, name="s3in")
        nc.gpsimd.memset(s2in[:, L_E1:L_E1 + PAD], 0.0)
        nc.gpsimd.memset(s3in[:, L_E2:L_E2 + PAD], 0.0)

        def stim_s1(g):
            if MODE["s1"] == "fp8":
                # rows (k,s) flat r=k*8+s; sub0 = r 0..11, sub1 = r 12..23,
                # at free offsets 0 / T.
                L = L_E1
                nc.gpsimd.dma_start(s1mov[0:8, 0:L],
                                    stim_in[g, :, 0:L])
                nc.gpsimd.dma_start(s1mov[8:12, 0:L],
                                    stim_in[g, 0:4, 1:1 + L])
                nc.gpsimd.dma_start(s1mov[0:4, T:T + L],
                                    stim_in[g, 4:8, 1:1 + L])
                nc.gpsimd.dma_start(s1mov[4:12, T:T + L],
                                    stim_in[g, :, 2:2 + L])
                Ws1dr = cload_pair("Ws1dr", 128)
                chs = _chunks(L_E1)
                for i in range(0, len(chs), 2):
                    grp = chs[i:i + 2]
                    ps = psC_p.tile([128, 2 * CH], F32, name="psconv",
                                    tag="psconv")
                    for j, (t0, w) in enumerate(grp):
                        nc.tensor.matmul(ps[:, j * CH:j * CH + w],
                                         Ws1dr,
                                         _dr_rhs(s1mov, t0, T, w, parts=12),
                                         perf_mode=DRM)
                    t0 = grp[0][0]
                    wtot = CH + grp[1][1] if len(grp) == 2 else grp[0][1]
                    evac_relu(s2in[:, t0:t0 + wtot], ps[:, :wtot],
                              bias["bias_s1"])
            else:
                # one casting load + two HWDGE shift copies (keeps SWDGE
                # descriptor-gen off the Pool engine)
                nc.gpsimd.dma_start(s1mov[0:8, 0:T], stim_in[g, :, :])
                nc.sync.dma_start(s1mov[8:16, 0:L_E1],
                                  s1mov[0:8, 1:1 + L_E1])
                nc.sync.dma_start(s1mov[16:24, 0:L_E1],
                                  s1mov[0:8, 2:2 + L_E1])
                Ws1 = cload("Ws1")
                chs = _chunks(L_E1)
                for i in range(0, len(chs), 2):
                    grp = chs[i:i + 2]
                    ps = psC_p.tile([128, 2 * CH], F32, name="psconv",
                                    tag="psconv")
                    for j, (t0, w) in enumerate(grp):
                        nc.tensor.matmul(ps[:, j * CH:j * CH + w], Ws1[:],
                                         s1mov[0:24, t0:t0 + w])
                    t0 = grp[0][0]
                    wtot = CH + grp[1][1] if len(grp) == 2 else grp[0][1]
                    evac_relu(s2in[:, t0:t0 + wtot], ps[:, :wtot],
                              bias["bias_s1"])

        # --- stim group machinery (pools outlive the eeg section so g0's
        # s3/norms/transpose can queue up work that overlaps the eeg DMA) ---
        stim_p = ctx.enter_context(tc.tile_pool(name="stimp", bufs=1))
        stT_p = ctx.enter_context(tc.tile_pool(name="stTp", bufs=2))

        def stim_group(g):
            # s3 conv into st_cm, st norms (DVE ttr), tail memset, stT
            # transpose.  The invns PE-transpose runs later (stim_finish)
            # to keep PSUM free for the eeg phase.
            st_cm = stim_p.tile([128, T], BF16, name="st_cm", bufs=2)
            conv_layer(s3in, st_cm, L_E3, 9, "s23", "Ws3", "bias_s3")
            ns2 = const_p.tile([128, 1], F32, name="ns2", tag=f"ns2_{g % 2}")
            ns4 = const_p.tile([128, 4], F32, name="ns4", tag=f"ns4_{g % 2}")
            qs = (L_E3 + 3) // 4
            for q in range(4):
                a, b = q * qs, min((q + 1) * qs, L_E3)
                if SQ_TTR:
                    nc.vector.tensor_tensor_reduce(
                        sqscr[:, 0:b - a], st_cm[:, a:b], st_cm[:, a:b],
                        1.0, 0.0, ALU.mult, ALU.add,
                        accum_out=ns4[:, q:q + 1])
                else:
                    nc.scalar.activation(sqscr[:, 0:b - a], st_cm[:, a:b],
                                         AF.Square,
                                         accum_out=ns4[:, q:q + 1])
            nc.vector.tensor_reduce(ns2[:], ns4[:],
                                    mybir.AxisListType.X, ALU.add)
            inv_ns = const_p.tile([128, 1], F32, name="inv_ns",
                                  tag=f"invns_{g % 2}")
            nc.scalar.sqrt(inv_ns[:], ns2[:])
            nc.vector.tensor_scalar_max(inv_ns[:], inv_ns[:], EPS)
            nc.vector.reciprocal(inv_ns[:], inv_ns[:])
            nc.gpsimd.memset(st_cm[:, L_E3:T], 0.0)
            stT = stT_p.tile([128, T // 128, 128], BF16, name="stT")
            for qq in range(4):
                nc.sync.dma_start_transpose(
                    stT[:, qq * 16:(qq + 1) * 16, :],
                    st_cm[:, qq * (T // 4):(qq + 1) * (T // 4)])
            return (g, stT, inv_ns)

        stim_s1(0)
        conv_layer(s2in, s3in, L_E2, 3, "s23", "Ws2", "bias_s2")
        grp0 = stim_group(0)
        stim_s1(1)
        conv_layer(s2in, s3in, L_E2, 3, "s23", "Ws2", "bias_s2")

        # ================= EEG path =================
        with ExitStack() as ectx:
            eeg_p = ectx.enter_context(tc.tile_pool(name="eegp", bufs=1))
            rot_p = ectx.enter_context(tc.tile_pool(name="eegrot", bufs=4))
            psT_p = ectx.enter_context(tc.tile_pool(name="psT", bufs=2,
                                                    space="PSUM"))

            e2in = eeg_p.tile([128, L_E1 + PAD], e23_dt, name="e2in")
            e3in = eeg_p.tile([128, L_E2 + PAD], e23_dt, name="e3in")
            nc.gpsimd.memset(e2in[:, L_E1:L_E1 + PAD], 0.0)
            nc.gpsimd.memset(e3in[:, L_E2:L_E2 + PAD], 0.0)

            TB = 2048
            PT = T + PAD
            chs_e1 = _chunks(L_E1)
            for duo in range(2):
                eegT = {}
                if MODE["e1"] == "q8":
                    eegT2 = eeg_p.tile([128, 2 * PT], e1_dt, name="eegT2",
                                       tag="eegT2")
                for lp in range(2):
                    p = 2 * duo + lp
                    if MODE["e1"] == "q8":
                        eegT_p = eegT2[:, lp * PT:(lp + 1) * PT]
                        nc.gpsimd.memset(eegT2[:, lp * PT + T:(lp + 1) * PT],
                                         0.0)
                    else:
                        eegT_p = eeg_p.tile([128, T + PAD], e1_dt,
                                            name="eegT", tag=f"eegT_{lp}")
                        nc.gpsimd.memset(eegT_p[:, T:T + PAD], 0.0)
                    eegT[lp] = eegT_p
                    for tb in range(T // TB):
                        ebf = rot_p.tile([128, TB // 128, 2, 64], BF16,
                                         name="ebf")
                        for dlt in range(2):
                            srcd = eeg_in[2 * p + dlt,
                                          tb * TB:(tb + 1) * TB, :]
                            nc.gpsimd.dma_start(
                                ebf[:, :, dlt, :],
                                srcd.rearrange("(th tl) c -> tl th c",
                                               tl=128))
                        for qb in range(TB // (2 * CH)):
                            psT = psT_p.tile([128, 8, 128], BF16,
                                             name="psT")
                            for u in range(8):
                                nc.tensor.matmul(psT[:, u, :],
                                                 ebf[:, qb * 8 + u, :, :],
                                                 id128[:],
                                                 is_transpose=True,
                                                 start=(u == 0),
                                                 stop=(u == 7))
                            t0 = tb * TB + qb * 2 * CH
                            evac_copy(eegT_p[:, t0:t0 + 2 * CH], psT[:])
                # fused conv1x1+e1
                if MODE["e1"] == "q8":
                    # per-tap DoubleRow: k-subtiles are the 2 seq-pair
                    # planes of eegT2; weights block-structured so sub j
                    # feeds out cols 32j..32j+31.
                    Wf1dr = [cload_pair(f"Wf1dr{k}", 64) for k in range(3)]
                    for i in range(0, len(chs_e1), 2):
                        grp = chs_e1[i:i + 2]
                        t0 = grp[0][0]
                        wtot = (CH + grp[1][1] if len(grp) == 2
                                else grp[0][1])
                        ps = psC_p.tile([128, 2 * CH], F32, name="pse1",
                                        tag="psconv")
                        for k in range(3):
                            for j, (tj, w) in enumerate(grp):
                                nc.tensor.matmul(
                                    ps[0:64, j * CH:j * CH + w],
                                    Wf1dr[k],
                                    _dr_rhs(eegT2, tj + k, PT, w),
                                    start=(k == 0), stop=(k == 2),
                                    perf_mode=DRM)
                        r0 = 64 * duo
                        evac_relu(e2in[r0:r0 + 64, t0:t0 + wtot],
                                  ps[0:64, :wtot],
                                  bias["bias_e1"][r0:r0 + 64])
                else:
                    Wf1 = [cload(f"Wf1_{k}") for k in range(3)]
                    for i in range(0, len(chs_e1), 2):
                        grp = chs_e1[i:i + 2]
                        t0 = grp[0][0]
                        wtot = (CH + grp[1][1] if len(grp) == 2
                                else grp[0][1])
                        for lp in range(2):
                            ps = psC_p.tile([128, 2 * CH], F32, name="pse1",
                                            tag="psconv")
                            for k in range(3):
                                for j, (tj, w) in enumerate(grp):
                                    nc.tensor.matmul(
                                        ps[32 * lp:32 * lp + 32,
                                           j * CH:j * CH + w],
                                        Wf1[k][:],
                                        eegT[lp][:, tj + k:tj + k + w],
                                        start=(k == 0), stop=(k == 2),
                                        tile_position=(0, 32 * lp))
                            r0 = 64 * duo + 32 * lp
                            evac_relu(e2in[r0:r0 + 32, t0:t0 + wtot],
                                      ps[32 * lp:32 * lp + 32, :wtot],
                                      bias["bias_e1"][r0:r0 + 32])

            conv_layer(e2in, e3in, L_E2, 3, "e23", "We2", "bias_e2")
            conv_layer(e3in, xf, L_E3, 9, "e23", "We3", "bias_e3")

        # ================= similarity path =================
        with ExitStack() as sctx:
            psD_p = sctx.enter_context(tc.tile_pool(name="psD", bufs=1,
                                                    space="PSUM"))
            psF_p = sctx.enter_context(tc.tile_pool(name="psF", bufs=1,
                                                    space="PSUM"))

            # x norms (xf ready post-eeg); 1/|x| goes into nxr row and is
            # applied after the dot via psB, so xf stays raw
            nx2 = const_p.tile([128, 1], F32, name="nx2")
            nx4 = const_p.tile([128, 4], F32, name="nx4")
            qs = (L_E3 + 3) // 4
            for q in range(4):
                a, b = q * qs, min((q + 1) * qs, L_E3)
                nc.scalar.activation(sqscr[:, 0:b - a], xf[:, a:b],
                                     AF.Square, accum_out=nx4[:, q:q + 1])
            nc.vector.tensor_reduce(nx2[:], nx4[:],
                                    mybir.AxisListType.X, ALU.add)
            nc.scalar.sqrt(inv_nx[:], nx2[:])
            nc.vector.tensor_scalar_max(inv_nx[:], inv_nx[:], EPS)
            nc.vector.reciprocal(inv_nx[:], inv_nx[:])
            psNx = psF_p.tile([1, 128], F32, name="psNx", tag="psN")
            nc.tensor.matmul(psNx[:], inv_nx[:], id128f[:],
                             is_transpose=True)
            nc.vector.tensor_copy(nxr[:], psNx[:])
            nc.gpsimd.memset(xf[:, L_E3:T], 0.0)
            for qq in range(4):
                nc.sync.dma_start_transpose(
                    xT[:, qq * 16:(qq + 1) * 16, :],
                    xf[:, qq * (T // 4):(qq + 1) * (T // 4)])

            def stim_finish(g, stT, inv_ns):
                psN = psF_p.tile([1, 128], F32, name="psN", tag="psN")
                nc.tensor.matmul(psN[:], inv_ns[:], id128f[:],
                                 is_transpose=True)
                invns_row = const_p.tile([1, 128], F32, name="invns_row",
                                         tag=f"invrow_{g % 2}")
                nc.vector.tensor_copy(invns_row[:], psN[:])
                return (g, stT, invns_row)

            def emit_dot(g, stT, invns_row):
                # stationary = stT chunk (ldweights is free), moving = the
                # 16 x-columns of batch g: out [128 si, 16 j] costs 16
                # cycles/chunk instead of 128.
                dot_ps = psD_p.tile([128, 16], F32, name="dot_ps")
                NCHK = T // 128
                for c in range(NCHK):
                    nc.tensor.matmul(dot_ps[:], stT[:, c, :],
                                     xT[:, c, g * 16:(g + 1) * 16],
                                     start=(c == 0), stop=(c == NCHK - 1))
                f1 = const_p.tile([128, 16], F32, name="f1",
                                  tag=f"f1_{g % 2}")
                nc.vector.tensor_mul(f1[:], dot_ps[:], W2c2[:])
                # psB[si, j] = inv_ns[si] * inv_nx[g*16+j]
                psB = psF_p.tile([128, 16], F32, name="psB", tag="psN")
                nc.tensor.matmul(psB[:], invns_row[:],
                                 nxr[0:1, g * 16:(g + 1) * 16])
                nc.vector.tensor_mul(f1[:], f1[:], psB[:])
                f3 = const_p.tile([128, 1], F32, name="f3",
                                  tag=f"f3_{g % 2}")
                nc.vector.tensor_reduce(f3[:], f1[:],
                                        mybir.AxisListType.X, ALU.add)
                fin_ps = psF_p.tile([1, S], F32, name="fin_ps", tag="psN")
                nc.tensor.matmul(fin_ps[:], f3[:], ones_blk[:])
                nc.vector.tensor_scalar_add(
                    out_sb[0:1, g * S:(g + 1) * S], fin_ps[:],
                    blin[0:1, 0:1])

            pending = [stim_finish(*grp0)]
            for g in range(1, BPC):
                gg = stim_group(g)
                if g < BPC - 1:
                    stim_s1(g + 1)
                    conv_layer(s2in, s3in, L_E2, 3, "s23", "Ws2", "bias_s2")
                pending.append(stim_finish(*gg))
                emit_dot(*pending.pop(0))
            emit_dot(*pending.pop(0))

        nc.sync.dma_start(out_dram[:], out_sb[:])


def _build(reps=1):
    nc = bass.Bass()
    dram = {
        "eeg_in": nc.dram_tensor("eeg_in", [BPC, T, C_EEG], F32,
                                 kind="ExternalInput"),
        "stim_in": nc.dram_tensor("stim_in", [BPC, S, T], F32,
                                  kind="ExternalInput"),
    }
    lay, offs = _blob_layout()
    for key, dt, dname in (("bf", BF16, "blob_bf"),
                           ("f32", F32, "blob_f32"),
                           ("f8", FP8, "blob_f8")):
        if offs[key]:
            dram[dname] = nc.dram_tensor(dname, [128, offs[key]], dt,
                                         kind="ExternalInput")
    dram["out"] = nc.dram_tensor("out", [1, BPC * S], F32,
                                 kind="ExternalOutput")

    with tile.TileContext(nc) as tc:
        if reps == 1:
            _build_body(nc, tc, dram)
        else:
            with tc.For_i(0, reps, 1):
                _build_body(nc, tc, dram)
    _split_multi_waits(nc)
    return nc


def _make_consts(inp):
    bf = ml_dtypes.bfloat16
    e4 = ml_dtypes.float8_e4m3
    c = {}
    w_eeg = np.asarray(inp["w_eeg"], np.float32)      # [8, 64, 1]

    def blockdiag(w, n_seq, ci, co):
        out = []
        for k in range(3):
            m = np.zeros((n_seq * ci, n_seq * co), np.float32)
            for s in range(n_seq):
                m[s * ci:(s + 1) * ci, s * co:(s + 1) * co] = w[:, :, k].T
            out.append(m)
        return out

    def pack_pairs(mats, name, dt):
        # mats: [W0, W1, W2] each [rows, M] f32 -> pairs (W0,W2), (W1,0)
        rows, M = mats[0].shape
        p0 = np.zeros((rows, 2 * M), np.float32)
        p0[:, 0:M] = mats[0]
        p0[:, M:2 * M] = mats[2]
        p1 = np.zeros((rows, 2 * M), np.float32)
        p1[:, 0:M] = mats[1]
        c[f"{name}p0"] = p0.astype(dt)
        c[f"{name}p1"] = p1.astype(dt)

    # fused conv1x1 + e1: Wf[co, c, k] = sum_ci w_e1[co,ci,k] * w_eeg[ci,c]
    w_e1 = np.asarray(inp["w_e1"], np.float32)
    if MODE["e1"] == "q8":
        for k in range(3):
            Mk = w_e1[:, :, k] @ w_eeg[:, :, 0]       # [16 co, 64 c]
            m = np.zeros((128, 2, 64), np.float32)
            for j in range(2):                        # k-subtile = seq pair
                for s in range(2):
                    m[s * 64:(s + 1) * 64, j,
                      32 * j + s * 16:32 * j + (s + 1) * 16] = Mk.T
            c[f"Wf1dr{k}"] = m.reshape(128, 128).astype(e4)
    else:
        for k in range(3):
            Mk = w_e1[:, :, k] @ w_eeg[:, :, 0]
            m = np.zeros((128, 32), np.float32)
            for s in range(2):
                m[s * 64:(s + 1) * 64, s * 16:(s + 1) * 16] = Mk.T
            c[f"Wf1_{k}"] = m.astype(bf)

    for pfx, mk, wns in (("We", "e23", ("w_e2", "w_e3")),
                         ("Ws", "s23", ("w_s2", "w_s3"))):
        for l, wn in zip((2, 3), wns):
            mats = blockdiag(np.asarray(inp[wn], np.float32), 8, 16, 16)
            if MODE[mk] == "q8":
                pack_pairs(mats, f"{pfx}{l}", e4)
            else:
                for k in range(3):
                    c[f"{pfx}{l}_{k}"] = mats[k].astype(bf)

    w_s1 = np.asarray(inp["w_s1"], np.float32)        # [16, 1, 3]
    Ws1 = np.zeros((24, 128), np.float32)
    for k in range(3):
        for s in range(8):
            Ws1[k * 8 + s, s * 16:(s + 1) * 16] = w_s1[:, 0, k]
    if MODE["s1"] == "fp8":
        # rows r=k*8+s: sub0 = rows 0..11, sub1 = rows 12..23
        W = np.zeros((12, 2 * 128), np.float32)
        W[:, 0:128] = Ws1[0:12]
        W[:, 128:256] = Ws1[12:24]
        c["Ws1dr"] = W.astype(e4)
    else:
        c["Ws1"] = Ws1.astype(bf)

    c["id128"] = np.eye(128, dtype=np.float32).astype(bf)
    c["id128f"] = np.eye(128, dtype=np.float32)
    b_e1f = (np.asarray(inp["b_e1"], np.float32) +
             sum(w_e1[:, :, k] for k in range(3))
             @ np.asarray(inp["b_eeg"], np.float32))
    c["bias_e1"] = np.tile(b_e1f, 8)[:, None]
    for n, srcn in (("bias_e2", "b_e2"),
                    ("bias_e3", "b_e3"), ("bias_s1", "b_s1"),
                    ("bias_s2", "b_s2"), ("bias_s3", "b_s3")):
        c[n] = np.tile(np.asarray(inp[srcn], np.float32), 8)[:, None]
    w_lin = np.asarray(inp["w_lin"], np.float32).reshape(16, 16)  # [i, j]
    c["W2c2"] = np.tile(w_lin, (8, 1))                 # [s*16+i, j]
    c["ones_blk"] = np.kron(np.eye(8, dtype=np.float32),
                            np.ones((16, 1), np.float32))
    c["blin"] = np.asarray(inp["b_lin"], np.float32).reshape(1, 1)

    lay, offs = _blob_layout()
    outb = {}
    npdt = {"bf": bf, "f32": np.float32, "f8": e4}
    for key, dname in (("bf", "blob_bf"), ("f32", "blob_f32"),
                       ("f8", "blob_f8")):
        if offs[key]:
            outb[dname] = np.zeros((128, offs[key]), npdt[key])
    for name, (which, off, shape) in lay.items():
        dname = {"bf": "blob_bf", "f32": "blob_f32", "f8": "blob_f8"}[which]
        outb[dname][0:shape[0], off:off + shape[1]] = c[name]
    return outb


def get_nc(reps=1):
    if reps not in _NC_CACHE:
        _NC_CACHE[reps] = _build(reps)
    return _NC_CACHE[reps]


def run(inputs, reps=1, trace=False):
    nc = get_nc(reps)
    consts = _make_consts(inputs)
    eeg = np.asarray(inputs["eeg"], np.float32)              # [64, 8192, 64]
    stim = np.asarray(inputs["stimulus"], np.float32)[..., 0]  # [64, 8, 8192]
    in_maps = []
    for ci in range(N_CORES):
        m = {"eeg_in": np.ascontiguousarray(eeg[ci * BPC:(ci + 1) * BPC]),
             "stim_in": np.ascontiguousarray(stim[ci * BPC:(ci + 1) * BPC])}
        m.update(consts)
        in_maps.append(m)
    res = run_bass_kernel_spmd(nc, in_maps, list(range(N_CORES)),
                               trace=trace)
    out = np.concatenate(
        [res.results[i]["out"].reshape(BPC, S) for i in range(N_CORES)],
        axis=0)
    return out.astype(np.float32)


def kernel(**inputs):
    return run(inputs, reps=1)


# revision 34
# speedup vs baseline: 1.0167x; 1.0167x over previous
"""Trainium2 Bass kernel for nn_DilatedConvModel (retrieval_knn).

Model: eeg [B,T,64] -> 1x1 conv (64->8) -> dilated conv stack (8->16->16->16,
dilations 1,3,9, VALID, relu); stimulus [B,S,T,1] -> dilated stack
(1->16->16->16); cosine similarity between all stim/eeg channel pairs over
time; 256->1 linear.  B=64, S=8, T=8192.

Sharding: pure data parallel over B across 8 cores (8 batches per core).

Per-core dataflow: channel-major convs on PE with block-diagonal weights
over the 8 local sequences.  Conv layers run as fp8e4m3 DoubleRow matmuls
(2 k-subtiles/instruction, 0.5 PE-cycles per moving column): the 3 taps are
packed as subtile pairs (tap0,tap2) [stride 2*dil, even as required] and
(tap1, zero-tap).  Activations are written as fp8 directly by the PSUM
evacuation (bias+relu) pass; buffer tails are memset so the zero-weight
junk reads stay finite.  Cosine dot products and norms stay bf16/f32.
"""

from contextlib import ExitStack

import numpy as np
import ml_dtypes

import concourse.bass as bass
import concourse.tile as tile
from concourse import mybir
from concourse.bass_utils import run_bass_kernel_spmd
from concourse.vector_clock import ScopedClock

# ---------------------------------------------------------------------------
# Workaround for walrus in this container rejecting >1 sync wait per
# instruction ("Too many sync wait commands"): (a) distribute the
# TileContext tail drain's sem waits across sync-engine nops, (b) post-pass
# that hoists extra waits of any instruction onto standalone EventSemaphore
# instructions inserted just before it on the same engine.
# ---------------------------------------------------------------------------
_MAX_WAITS = 1


def _patched_drain_and_barrier(self, tick_clock, wait_clock):
    nc = self.nc
    probe = nc.sync.nop()
    wait_clock.add_sem_waits(probe.ins,
                             ScopedClock({None: tick_clock.global_clock}))
    si = probe.ins.sync_info
    waits = list(si.on_wait) if si and si.on_wait else []
    if len(waits) > _MAX_WAITS:
        si.on_wait = waits[:_MAX_WAITS]
        rest = waits[_MAX_WAITS:]
        while rest:
            extra = nc.sync.nop()
            extra.ins.sync_info = mybir.SyncInfo(on_wait=rest[:_MAX_WAITS],
                                                 on_update=[])
            rest = rest[_MAX_WAITS:]
    # probe/extra nops precede the drain in SP program order, so the drain
    # only runs after every lane's final tick — no waits needed on it.
    nc.sync.drain()
    nc.all_engine_barrier()
    assert self.sems is not None
    popped = nc._tile_sem_poison_stack.pop()
    assert popped is self._sem_poison
    nc.clear_and_free_semaphores(list(self.sems.allocated().values()))
    nc.all_engine_barrier()


def _split_multi_waits(nc, max_waits=_MAX_WAITS):
    f = nc.m.functions[0]
    ctr = 0
    for bb in f.blocks:
        new_insts = []
        for inst in bb.instructions:
            si = inst.sync_info
            waits = list(si.on_wait) if si and si.on_wait else []
            if len(waits) > max_waits:
                for w in waits[:-max_waits]:
                    ev = mybir.InstEventSemaphore(
                        name=f"waitsplit_{ctr}", opcode="EventSemaphore",
                        engine=inst.engine, ins=[], outs=[],
                        sync_info=mybir.SyncInfo(on_wait=[w], on_update=[]))
                    ctr += 1
                    new_insts.append(ev)
                si.on_wait = waits[-max_waits:]
            new_insts.append(inst)
        try:
            bb.instructions[:] = new_insts
        except TypeError:
            bb.instructions = new_insts


tile.TileContext._drain_and_barrier = _patched_drain_and_barrier

BF16 = mybir.dt.bfloat16
F32 = mybir.dt.float32
FP8 = mybir.dt.float8e4
AF = mybir.ActivationFunctionType
ALU = mybir.AluOpType
DRM = mybir.MatmulPerfMode.DoubleRow

B, S, T, C_EEG = 64, 8, 8192, 64
N_CORES = 8
BPC = B // N_CORES          # 8 sequences per core
CH = 512                    # fp32 PSUM chunk width
PAD = 64                    # activation tile tail pad (junk reads of P1 sub1)
L_C1, L_E1, L_E2, L_E3 = 8192, 8190, 8184, 8166
EPS = 1e-8

# per-layer precision: "q8" = fp8 DoubleRow (1.0 cyc/col/layer),
# "bf16" = 3-tap bf16 (3.0 cyc/col/layer).  s1: "fp8" (0.5) or "bf16" (1.0).
# The stim path must stay bf16: fp8 perturbs the st3 direction ~3%
# coherently, which multiplies every cosine (measured 2-3e-2 rel err).
MODE = {"e1": "q8", "e23": "q8", "s1": "bf16", "s23": "bf16"}
SQ_TTR = False

_NC_CACHE = {}


def _act_dt(mode):
    return FP8 if mode in ("q8",) else BF16


def _chunks(length):
    out, t0 = [], 0
    while t0 < length:
        w = min(CH, length - t0)
        out.append((t0, w))
        t0 += w
    return out


def _dr_rhs(src_m, t0, stride, width, parts=128):
    """[parts, 2, width] view of flat [parts, T] tile: subtiles at t0 and
    t0+stride (stride must be even for fp8 DoubleRow)."""
    ap = src_m[0:parts, t0:t0 + 1].copy()
    v = ap.ap
    part = tuple(v[0])
    v.clear()
    v.append(part)
    v.append((stride, 2))
    v.append((1, width))
    return ap


def _const_shapes():
    d = {
        "id128": ((128, 128), BF16),
        "id128f": ((128, 128), F32),
        "W2c2": ((128, 16), F32),
        "ones_blk": ((128, 8), F32),
        "blin": ((1, 1), F32),
    }
    # s1 weights
    if MODE["s1"] == "fp8":
        d["Ws1dr"] = ((12, 2 * 128), FP8)
    else:
        d["Ws1"] = ((24, 128), BF16)
    # e1 fused weights: q8 = per-tap DoubleRow over 2 seq-pair k-subtiles
    # (4 seqs, 64 out cols); bf16 = 2-seq packed 32-col strips
    if MODE["e1"] == "q8":
        for k in range(3):
            d[f"Wf1dr{k}"] = ((128, 2 * 64), FP8)
    else:
        for k in range(3):
            d[f"Wf1_{k}"] = ((128, 32), BF16)
    for pfx, mk in (("We", "e23"), ("Ws", "s23")):
        if MODE[mk] == "q8":
            for l in (2, 3):
                for pi in range(2):
                    d[f"{pfx}{l}p{pi}"] = ((128, 2 * 128), FP8)
        else:
            for l in (2, 3):
                for k in range(3):
                    d[f"{pfx}{l}_{k}"] = ((128, 128), BF16)
    for n in ("bias_e1", "bias_e2", "bias_e3", "bias_s1", "bias_s2",
              "bias_s3"):
        d[n] = ((128, 1), F32)
    return d


def _blob_layout():
    """column layout of consts inside the three blobs (rows, cols, dtype)"""
    items = {"bf": [], "f32": [], "f8": []}
    for name, (shape, dt) in _const_shapes().items():
        key = "bf" if dt == BF16 else ("f32" if dt == F32 else "f8")
        items[key].append((name, shape))
    lay, offs = {}, {"bf": 0, "f32": 0, "f8": 0}
    for key in ("bf", "f32", "f8"):
        for name, shape in items[key]:
            lay[name] = (key, offs[key], shape)
            offs[key] += shape[1]
    return lay, offs


def _build_body(nc, tc, dram):
    eeg_in, stim_in, out_dram = dram["eeg_in"], dram["stim_in"], dram["out"]

    with ExitStack() as ctx:
        const_p = ctx.enter_context(tc.tile_pool(name="const", bufs=1))
        persist_p = ctx.enter_context(tc.tile_pool(name="persist", bufs=1))
        early_p = ctx.enter_context(tc.tile_pool(name="early", bufs=1))
        psC_p = ctx.enter_context(tc.tile_pool(name="psC", bufs=3,
                                               space="PSUM"))

        lay, offs = _blob_layout()
        blobs = {}
        for key, dt, dname in (("bf", BF16, "blob_bf"),
                               ("f32", F32, "blob_f32"),
                               ("f8", FP8, "blob_f8")):
            if offs[key]:
                bt = const_p.tile([128, offs[key]], dt, name=dname)
                nc.sync.dma_start(bt[:], dram[dname][:])
                blobs[key] = bt

        def cload(name):
            which, off, shape = lay[name]
            return blobs[which][0:shape[0], off:off + shape[1]]

        def cload_pair(name, m):
            which, off, shape = lay[name]
            return blobs[which][0:shape[0], off:off + shape[1]].rearrange(
                "p (a m) -> p a m", a=2)

        id128 = cload("id128")
        id128f = cload("id128f")
        bias = {n: cload(n) for n in
                ("bias_e1", "bias_e2", "bias_e3",
                 "bias_s1", "bias_s2", "bias_s3")}
        W2c2 = cload("W2c2")
        ones_blk = cload("ones_blk")
        blin = cload("blin")

        out_sb = const_p.tile([1, BPC * S], F32, name="out_sb")
        inv_nx = const_p.tile([128, 1], F32, name="inv_nx")
        sqscr = const_p.tile([128, 1024], BF16, name="sqscr")

        xT = persist_p.tile([128, T // 128, 128], BF16, name="xT")
        xf = persist_p.tile([128, T], BF16, name="xf")

        evac_ctr = [0]
        EVAC_PAT = "ADADADD"    # ACT:DVE = 3:4 (Pool cannot read PSUM)

        def evac_relu(dst, src, bias_t):
            c = EVAC_PAT[evac_ctr[0] % len(EVAC_PAT)]
            if c == "A":
                nc.scalar.activation(dst, src, AF.Relu, bias=bias_t[:, 0:1])
            elif c == "D":
                nc.vector.tensor_scalar(dst, src, bias_t[:, 0:1], 0.0,
                                        ALU.add, ALU.max)
            else:
                nc.gpsimd.tensor_scalar(dst, src, bias_t[:, 0:1], 0.0,
                                        ALU.add, ALU.max)
            evac_ctr[0] += 1

        def evac_copy(dst, src):
            c = EVAC_PAT[evac_ctr[0] % len(EVAC_PAT)]
            if c == "A":
                nc.scalar.copy(dst, src)
            elif c == "D":
                nc.vector.tensor_copy(dst, src)
            else:
                nc.gpsimd.tensor_copy(dst, src)
            evac_ctr[0] += 1

        def conv_layer_bf16(src_m, rows, dst_m, out_len, dil, Wk, bn,
                            post=None):
            chs = _chunks(out_len)
            for i in range(0, len(chs), 2):
                grp = chs[i:i + 2]
                ps = psC_p.tile([128, 2 * CH], F32, name="psconv",
                                tag="psconv")
                for k in range(3):
                    for j, (t0, w) in enumerate(grp):
                        nc.tensor.matmul(
                            ps[:, j * CH:j * CH + w], Wk[k][:],
                            src_m[0:rows, t0 + k * dil:t0 + k * dil + w],
                            start=(k == 0), stop=(k == 2))
                t0 = grp[0][0]
                wtot = CH + grp[1][1] if len(grp) == 2 else grp[0][1]
                evac_relu(dst_m[:, t0:t0 + wtot], ps[:, :wtot], bias[bn])
                if post is not None:
                    post(t0 + wtot)

        def conv_layer_dr(src_m, dst_m, out_len, dil, Wpairs, bn):
            # Wpairs: [(Wpair_ap, tap_offset), ...]; each a DoubleRow over
            # subtile stride 2*dil.
            chs = _chunks(out_len)
            npi = len(Wpairs)
            for i in range(0, len(chs), 2):
                grp = chs[i:i + 2]
                ps = psC_p.tile([128, 2 * CH], F32, name="psconv",
                                tag="psconv")
                for pi, (Wp, off) in enumerate(Wpairs):
                    for j, (t0, w) in enumerate(grp):
                        nc.tensor.matmul(
                            ps[:, j * CH:j * CH + w], Wp,
                            _dr_rhs(src_m, t0 + off, 2 * dil, w),
                            start=(pi == 0), stop=(pi == npi - 1),
                            perf_mode=DRM)
                t0 = grp[0][0]
                wtot = CH + grp[1][1] if len(grp) == 2 else grp[0][1]
                evac_relu(dst_m[:, t0:t0 + wtot], ps[:, :wtot], bias[bn])

        def conv_layer(src_m, dst_m, out_len, dil, mk, wname, bn,
                       post=None):
            if MODE[mk] == "q8":
                Wp = [(cload_pair(f"{wname}p0", 128), 0),
                      (cload_pair(f"{wname}p1", 128), dil)]
                conv_layer_dr(src_m, dst_m, out_len, dil, Wp, bn)
                if post is not None:
                    post(out_len)
            else:
                Wk = [cload(f"{wname}_{k}") for k in range(3)]
                conv_layer_bf16(src_m, 128, dst_m, out_len, dil, Wk, bn,
                                post=post)

        s23_dt = _act_dt(MODE["s23"])
        e23_dt = _act_dt(MODE["e23"])
        e1_dt = _act_dt(MODE["e1"])

        # ---- early: stimulus group 0 s1+s2 (fills PE while eeg DMA runs)
        s1mov_t = early_p.tile([24, T], BF16, name="s1mov")

        def s1mov_tile(g):
            return s1mov_t
        s2in = early_p.tile([128, L_E1 + PAD], s23_dt, name="s2in")
        s3in = early_p.tile([128, L_E2 + PAD], s23_dt, name="s3in")
        nc.gpsimd.memset(s2in[:, L_E1:L_E1 + PAD], 0.0)
        nc.gpsimd.memset(s3in[:, L_E2:L_E2 + PAD], 0.0)

        def stim_s1_load(g):
            s1mov = s1mov_tile(g)
            for k in range(3):
                nc.gpsimd.dma_start(s1mov[k * 8:(k + 1) * 8, 0:L_E1],
                                    stim_in[g, :, k:k + L_E1])

        def stim_s1_mm(g):
            s1mov = s1mov_tile(g)
            Ws1 = cload("Ws1")
            chs = _chunks(L_E1)
            for i in range(0, len(chs), 2):
                grp = chs[i:i + 2]
                ps = psC_p.tile([128, 2 * CH], F32, name="psconv",
                                tag="psconv")
                for j, (t0, w) in enumerate(grp):
                    nc.tensor.matmul(ps[:, j * CH:j * CH + w], Ws1[:],
                                     s1mov[0:24, t0:t0 + w])
                t0 = grp[0][0]
                wtot = CH + grp[1][1] if len(grp) == 2 else grp[0][1]
                evac_relu(s2in[:, t0:t0 + wtot], ps[:, :wtot],
                          bias["bias_s1"])

        # --- stim group machinery (pools outlive the eeg section so g0's
        # s3/norms/transpose can queue up work that overlaps the eeg DMA) ---
        stim_p = ctx.enter_context(tc.tile_pool(name="stimp", bufs=1))
        stT_p = ctx.enter_context(tc.tile_pool(name="stTp", bufs=2))

        def stim_group(g):
            # s3 conv into st_cm, st norms (DVE ttr), tail memset, stT
            # transpose.  The invns PE-transpose runs later (stim_finish)
            # to keep PSUM free for the eeg phase.
            st_cm = stim_p.tile([128, T], BF16, name="st_cm", bufs=2)
            ns2 = const_p.tile([128, 1], F32, name="ns2", tag=f"ns2_{g % 2}")
            ns4 = const_p.tile([128, 8], F32, name="ns4", tag=f"ns4_{g % 2}")
            qs = (L_E3 + 7) // 8
            qdone = [0]

            def sq_post(covered):
                while qdone[0] < 8 and (qdone[0] + 1) * qs <= covered \
                        or (qdone[0] < 8 and covered >= L_E3):
                    q = qdone[0]
                    a, b = q * qs, min((q + 1) * qs, L_E3)
                    nc.scalar.activation(sqscr[:, 0:b - a], st_cm[:, a:b],
                                         AF.Square,
                                         accum_out=ns4[:, q:q + 1])
                    qdone[0] += 1

            conv_layer(s3in, st_cm, L_E3, 9, "s23", "Ws3", "bias_s3",
                       post=sq_post)
            assert qdone[0] == 8
            nc.vector.tensor_reduce(ns2[:], ns4[:],
                                    mybir.AxisListType.X, ALU.add)
            inv_ns = const_p.tile([128, 1], F32, name="inv_ns",
                                  tag=f"invns_{g % 2}")
            nc.scalar.sqrt(inv_ns[:], ns2[:])
            nc.vector.tensor_scalar_max(inv_ns[:], inv_ns[:], EPS)
            nc.vector.reciprocal(inv_ns[:], inv_ns[:])
            nc.gpsimd.memset(st_cm[:, L_E3:T], 0.0)
            stT = stT_p.tile([128, T // 128, 128], BF16, name="stT")
            for qq in range(4):
                nc.sync.dma_start_transpose(
                    stT[:, qq * 16:(qq + 1) * 16, :],
                    st_cm[:, qq * (T // 4):(qq + 1) * (T // 4)])
            return (g, stT, inv_ns)

        stim_s1_load(0)
        stim_s1_mm(0)
        conv_layer(s2in, s3in, L_E2, 3, "s23", "Ws2", "bias_s2")
        stim_s1_load(1)
        grp0 = stim_group(0)
        stim_s1_mm(1)
        conv_layer(s2in, s3in, L_E2, 3, "s23", "Ws2", "bias_s2")
        stim_s1_load(2)

        # ================= EEG path =================
        with ExitStack() as ectx:
            eeg_p = ectx.enter_context(tc.tile_pool(name="eegp", bufs=1))
            rot_p = ectx.enter_context(tc.tile_pool(name="eegrot", bufs=2 if MODE["e23"] == "bf16" else 3))
            psT_p = ectx.enter_context(tc.tile_pool(name="psT", bufs=2,
                                                    space="PSUM"))

            e2in = eeg_p.tile([128, L_E1 + PAD], e23_dt, name="e2in")
            e3in = eeg_p.tile([128, L_E2 + PAD], e23_dt, name="e3in")
            nc.gpsimd.memset(e2in[:, L_E1:L_E1 + PAD], 0.0)
            nc.gpsimd.memset(e3in[:, L_E2:L_E2 + PAD], 0.0)

            TB = 2048
            PT = T + PAD
            chs_e1 = _chunks(L_E1)
            for duo in range(2):
                eegT = {}
                if MODE["e1"] == "q8":
                    eegT2 = eeg_p.tile([128, 2 * PT], e1_dt, name="eegT2",
                                       tag="eegT2")
                for lp in range(2):
                    p = 2 * duo + lp
                    if MODE["e1"] == "q8":
                        eegT_p = eegT2[:, lp * PT:(lp + 1) * PT]
                        nc.gpsimd.memset(eegT2[:, lp * PT + T:(lp + 1) * PT],
                                         0.0)
                    else:
                        eegT_p = eeg_p.tile([128, T + PAD], e1_dt,
                                            name="eegT", tag=f"eegT_{lp}")
                        nc.gpsimd.memset(eegT_p[:, T:T + PAD], 0.0)
                    eegT[lp] = eegT_p
                    for tb in range(T // TB):
                        ebf = rot_p.tile([128, TB // 128, 2, 64], BF16,
                                         name="ebf")
                        for dlt in range(2):
                            srcd = eeg_in[2 * p + dlt,
                                          tb * TB:(tb + 1) * TB, :]
                            nc.gpsimd.dma_start(
                                ebf[:, :, dlt, :],
                                srcd.rearrange("(th tl) c -> tl th c",
                                               tl=128))
                        for qb in range(TB // (2 * CH)):
                            psT = psT_p.tile([128, 8, 128], BF16,
                                             name="psT")
                            for u in range(8):
                                nc.tensor.matmul(psT[:, u, :],
                                                 ebf[:, qb * 8 + u, :, :],
                                                 id128[:],
                                                 is_transpose=True,
                                                 start=(u == 0),
                                                 stop=(u == 7))
                            t0 = tb * TB + qb * 2 * CH
                            evac_copy(eegT_p[:, t0:t0 + 2 * CH], psT[:])
                # fused conv1x1+e1
                if MODE["e1"] == "q8":
                    # per-tap DoubleRow: k-subtiles are the 2 seq-pair
                    # planes of eegT2; weights block-structured so sub j
                    # feeds out cols 32j..32j+31.
                    Wf1dr = [cload_pair(f"Wf1dr{k}", 64) for k in range(3)]
                    for i in range(0, len(chs_e1), 2):
                        grp = chs_e1[i:i + 2]
                        t0 = grp[0][0]
                        wtot = (CH + grp[1][1] if len(grp) == 2
                                else grp[0][1])
                        ps = psC_p.tile([128, 2 * CH], F32, name="pse1",
                                        tag="psconv")
                        for k in range(3):
                            for j, (tj, w) in enumerate(grp):
                                nc.tensor.matmul(
                                    ps[0:64, j * CH:j * CH + w],
                                    Wf1dr[k],
                                    _dr_rhs(eegT2, tj + k, PT, w),
                                    start=(k == 0), stop=(k == 2),
                                    perf_mode=DRM)
                        r0 = 64 * duo
                        evac_relu(e2in[r0:r0 + 64, t0:t0 + wtot],
                                  ps[0:64, :wtot],
                                  bias["bias_e1"][r0:r0 + 64])
                else:
                    Wf1 = [cload(f"Wf1_{k}") for k in range(3)]
                    for i in range(0, len(chs_e1), 2):
                        grp = chs_e1[i:i + 2]
                        t0 = grp[0][0]
                        wtot = (CH + grp[1][1] if len(grp) == 2
                                else grp[0][1])
                        for lp in range(2):
                            ps = psC_p.tile([128, 2 * CH], F32, name="pse1",
                                            tag="psconv")
                            for k in range(3):
                                for j, (tj, w) in enumerate(grp):
                                    nc.tensor.matmul(
                                        ps[32 * lp:32 * lp + 32,
                                           j * CH:j * CH + w],
                                        Wf1[k][:],
                                        eegT[lp][:, tj + k:tj + k + w],
                                        start=(k == 0), stop=(k == 2),
                                        tile_position=(0, 32 * lp))
                            r0 = 64 * duo + 32 * lp
                            evac_relu(e2in[r0:r0 + 32, t0:t0 + wtot],
                                      ps[32 * lp:32 * lp + 32, :wtot],
                                      bias["bias_e1"][r0:r0 + 32])

            conv_layer(e2in, e3in, L_E2, 3, "e23", "We2", "bias_e2")
            conv_layer(e3in, xf, L_E3, 9, "e23", "We3", "bias_e3")

        # ================= similarity path =================
        with ExitStack() as sctx:
            psD_p = sctx.enter_context(tc.tile_pool(name="psD", bufs=1,
                                                    space="PSUM"))
            psF_p = sctx.enter_context(tc.tile_pool(name="psF", bufs=1,
                                                    space="PSUM"))

            # x norms (xf ready post-eeg); 1/|x| goes into nxr row and is
            # applied after the dot via psB, so xf stays raw
            nx2 = const_p.tile([128, 1], F32, name="nx2")
            nx4 = const_p.tile([128, 8], F32, name="nx4")
            qs = (L_E3 + 7) // 8
            for q in range(8):
                a, b = q * qs, min((q + 1) * qs, L_E3)
                nc.scalar.activation(sqscr[:, 0:b - a], xf[:, a:b],
                                     AF.Square, accum_out=nx4[:, q:q + 1])
            nc.vector.tensor_reduce(nx2[:], nx4[:],
                                    mybir.AxisListType.X, ALU.add)
            nc.scalar.sqrt(inv_nx[:], nx2[:])
            nc.vector.tensor_scalar_max(inv_nx[:], inv_nx[:], EPS)
            nc.vector.reciprocal(inv_nx[:], inv_nx[:])
            nc.vector.tensor_scalar_mul(xf[:, :L_E3], xf[:, :L_E3],
                                        inv_nx[:, 0:1])
            nc.gpsimd.memset(xf[:, L_E3:T], 0.0)
            for qq in range(4):
                nc.sync.dma_start_transpose(
                    xT[:, qq * 16:(qq + 1) * 16, :],
                    xf[:, qq * (T // 4):(qq + 1) * (T // 4)])

            def emit_dot(g, stT, inv_ns):
                # stationary = stT chunk (ldweights is free), moving = the
                # 16 x-columns of batch g: out [128 si, 16 j] costs 16
                # cycles/chunk instead of 128.
                dot_ps = psD_p.tile([128, 16], F32, name="dot_ps")
                NCHK = T // 128
                for c in range(NCHK):
                    nc.tensor.matmul(dot_ps[:], stT[:, c, :],
                                     xT[:, c, g * 16:(g + 1) * 16],
                                     start=(c == 0), stop=(c == NCHK - 1))
                f1 = const_p.tile([128, 16], F32, name="f1",
                                  tag=f"f1_{g % 2}")
                nc.vector.tensor_mul(f1[:], dot_ps[:], W2c2[:])
                nc.vector.tensor_scalar_mul(f1[:], f1[:], inv_ns[:, 0:1])
                f3 = const_p.tile([128, 1], F32, name="f3",
                                  tag=f"f3_{g % 2}")
                nc.vector.tensor_reduce(f3[:], f1[:],
                                        mybir.AxisListType.X, ALU.add)
                fin_ps = psF_p.tile([1, S], F32, name="fin_ps", tag="psN")
                nc.tensor.matmul(fin_ps[:], f3[:], ones_blk[:])
                nc.vector.tensor_scalar_add(
                    out_sb[0:1, g * S:(g + 1) * S], fin_ps[:],
                    blin[0:1, 0:1])

            pending = [grp0]
            for g in range(1, BPC):
                gg = stim_group(g)
                if g < BPC - 1:
                    stim_s1_mm(g + 1)
                    conv_layer(s2in, s3in, L_E2, 3, "s23", "Ws2", "bias_s2")
                if g < BPC - 2:
                    stim_s1_load(g + 2)
                pending.append(gg)
                emit_dot(*pending.pop(0))
            emit_dot(*pending.pop(0))

        nc.sync.dma_start(out_dram[:], out_sb[:])


def _build(reps=1):
    nc = bass.Bass()
    dram = {
        "eeg_in": nc.dram_tensor("eeg_in", [BPC, T, C_EEG], F32,
                                 kind="ExternalInput"),
        "stim_in": nc.dram_tensor("stim_in", [BPC, S, T], F32,
                                  kind="ExternalInput"),
    }
    lay, offs = _blob_layout()
    for key, dt, dname in (("bf", BF16, "blob_bf"),
                           ("f32", F32, "blob_f32"),
                           ("f8", FP8, "blob_f8")):
        if offs[key]:
            dram[dname] = nc.dram_tensor(dname, [128, offs[key]], dt,
                                         kind="ExternalInput")
    dram["out"] = nc.dram_tensor("out", [1, BPC * S], F32,
                                 kind="ExternalOutput")

    with tile.TileContext(nc) as tc:
        if reps == 1:
            _build_body(nc, tc, dram)
        else:
            with tc.For_i(0, reps, 1):
                _build_body(nc, tc, dram)
    _split_multi_waits(nc)
    return nc


def _make_consts(inp):
    bf = ml_dtypes.bfloat16
    e4 = ml_dtypes.float8_e4m3
    c = {}
    w_eeg = np.asarray(inp["w_eeg"], np.float32)      # [8, 64, 1]

    def blockdiag(w, n_seq, ci, co):
        out = []
        for k in range(3):
            m = np.zeros((n_seq * ci, n_seq * co), np.float32)
            for s in range(n_seq):
                m[s * ci:(s + 1) * ci, s * co:(s + 1) * co] = w[:, :, k].T
            out.append(m)
        return out

    def pack_pairs(mats, name, dt):
        # mats: [W0, W1, W2] each [rows, M] f32 -> pairs (W0,W2), (W1,0)
        rows, M = mats[0].shape
        p0 = np.zeros((rows, 2 * M), np.float32)
        p0[:, 0:M] = mats[0]
        p0[:, M:2 * M] = mats[2]
        p1 = np.zeros((rows, 2 * M), np.float32)
        p1[:, 0:M] = mats[1]
        c[f"{name}p0"] = p0.astype(dt)
        c[f"{name}p1"] = p1.astype(dt)

    # fused conv1x1 + e1: Wf[co, c, k] = sum_ci w_e1[co,ci,k] * w_eeg[ci,c]
    w_e1 = np.asarray(inp["w_e1"], np.float32)
    if MODE["e1"] == "q8":
        for k in range(3):
            Mk = w_e1[:, :, k] @ w_eeg[:, :, 0]       # [16 co, 64 c]
            m = np.zeros((128, 2, 64), np.float32)
            for j in range(2):                        # k-subtile = seq pair
                for s in range(2):
                    m[s * 64:(s + 1) * 64, j,
                      32 * j + s * 16:32 * j + (s + 1) * 16] = Mk.T
            c[f"Wf1dr{k}"] = m.reshape(128, 128).astype(e4)
    else:
        for k in range(3):
            Mk = w_e1[:, :, k] @ w_eeg[:, :, 0]
            m = np.zeros((128, 32), np.float32)
            for s in range(2):
                m[s * 64:(s + 1) * 64, s * 16:(s + 1) * 16] = Mk.T
            c[f"Wf1_{k}"] = m.astype(bf)

    for pfx, mk, wns in (("We", "e23", ("w_e2", "w_e3")),
                         ("Ws", "s23", ("w_s2", "w_s3"))):
        for l, wn in zip((2, 3), wns):
            mats = blockdiag(np.asarray(inp[wn], np.float32), 8, 16, 16)
            if MODE[mk] == "q8":
                pack_pairs(mats, f"{pfx}{l}", e4)
            else:
                for k in range(3):
                    c[f"{pfx}{l}_{k}"] = mats[k].astype(bf)

    w_s1 = np.asarray(inp["w_s1"], np.float32)        # [16, 1, 3]
    Ws1 = np.zeros((24, 128), np.float32)
    for k in range(3):
        for s in range(8):
            Ws1[k * 8 + s, s * 16:(s + 1) * 16] = w_s1[:, 0, k]
    if MODE["s1"] == "fp8":
        # rows r=k*8+s: sub0 = rows 0..11, sub1 = rows 12..23
        W = np.zeros((12, 2 * 128), np.float32)
        W[:, 0:128] = Ws1[0:12]
        W[:, 128:256] = Ws1[12:24]
        c["Ws1dr"] = W.astype(e4)
    else:
        c["Ws1"] = Ws1.astype(bf)

    c["id128"] = np.eye(128, dtype=np.float32).astype(bf)
    c["id128f"] = np.eye(128, dtype=np.float32)
    b_e1f = (np.asarray(inp["b_e1"], np.float32) +
             sum(w_e1[:, :, k] for k in range(3))
             @ np.asarray(inp["b_eeg"], np.float32))
    c["bias_e1"] = np.tile(b_e1f, 8)[:, None]
    for n, srcn in (("bias_e2", "b_e2"),
                    ("bias_e3", "b_e3"), ("bias_s1", "b_s1"),
                    ("bias_s2", "b_s2"), ("bias_s3", "b_s3")):
        c[n] = np.tile(np.asarray(inp[srcn], np.float32), 8)[:, None]
    w_lin = np.asarray(inp["w_lin"], np.float32).reshape(16, 16)  # [i, j]
    c["W2c2"] = np.tile(w_lin, (8, 1))                 # [s*16+i, j]
    c["ones_blk"] = np.kron(np.eye(8, dtype=np.float32),
                            np.ones((16, 1), np.float32))
    c["blin"] = np.asarray(inp["b_lin"], np.float32).reshape(1, 1)

    lay, offs = _blob_layout()
    outb = {}
    npdt = {"bf": bf, "f32": np.float32, "f8": e4}
    for key, dname in (("bf", "blob_bf"), ("f32", "blob_f32"),
                       ("f8", "blob_f8")):
        if offs[key]:
            outb[dname] = np.zeros((128, offs[key]), npdt[key])
    for name, (which, off, shape) in lay.items():
        dname = {"bf": "blob_bf", "f32": "blob_f32", "f8": "blob_f8"}[which]
        outb[dname][0:shape[0], off:off + shape[1]] = c[name]
    return outb


def get_nc(reps=1):
    if reps not in _NC_CACHE:
        _NC_CACHE[reps] = _build(reps)
    return _NC_CACHE[reps]


def run(inputs, reps=1, trace=False):
    nc = get_nc(reps)
    consts = _make_consts(inputs)
    eeg = np.asarray(inputs["eeg"], np.float32)              # [64, 8192, 64]
    stim = np.asarray(inputs["stimulus"], np.float32)[..., 0]  # [64, 8, 8192]
    in_maps = []
    for ci in range(N_CORES):
        m = {"eeg_in": np.ascontiguousarray(eeg[ci * BPC:(ci + 1) * BPC]),
             "stim_in": np.ascontiguousarray(stim[ci * BPC:(ci + 1) * BPC])}
        m.update(consts)
        in_maps.append(m)
    res = run_bass_kernel_spmd(nc, in_maps, list(range(N_CORES)),
                               trace=trace)
    out = np.concatenate(
        [res.results[i]["out"].reshape(BPC, S) for i in range(N_CORES)],
        axis=0)
    return out.astype(np.float32)


def kernel(**inputs):
    return run(inputs, reps=1)
